# revision 1
# baseline (speedup 1.0000x reference)
"""Trainium2 Bass kernel for a 2-layer GCN + global mean pool + MLP head.

Distribution (8 NeuronCores): edge-parallel. Edges (plus one self-loop per
node) are sharded across cores as part of input distribution; each core
gathers node-table rows by src (dma_gather) and scatter-adds them by dst
(dma_scatter_add with SDMA CCE f32 add) into per-core partial accumulators;
node-boundary partial sums are combined with AllReduce. Small parameters are
replicated.

Math: with c = rsqrt(deg) (deg counts in-edges incl. the self loop), each
GCN layer is  h' = relu(c * (sum_{u->v} t[u]) + b)  with  t = c * (h @ W).
The layer-2 weight multiply commutes with the edge sum, so the second edge
pass scatters u1 = c * h1 rows and W2 is applied after the reduce. Column 32
of the u1 rows carries the constant 1, so acc2[:,32] reproduces deg and the
final phase is self-contained per gathered row.

Race-freedom: duplicate scatter destinations within one dma_scatter_add and
across concurrently-running ones are not accumulated correctly by the DMA
engines, so the host deals each (src-chunk, dst-chunk) edge segment into
bins with unique dst (rank-within-dst dealing), and all scatter instructions
that target the same dst-chunk accumulator are chained with explicit deps.
"""

import numpy as np

import concourse.bacc as bacc
import concourse.mybir as mybir
import concourse.tile as tile
from concourse.bass_utils import run_bass_kernel_spmd
from bass_rust import add_dep_helper

# ---- problem geometry (hardcoded per task contract) ----
N = 100000
E = 1000000
G = 256
NTYPES = 200
EMB = 64            # embedding dim; also the 256B table row width (64 f32)
HID = 32
C1 = 16
NCORES = 8

CH_REAL = 25600     # real node rows per chunk (int16-addressable)
CH_PAD = 32768      # chunk stride (16 * 2048)
NCHUNK = 4
NROW = NCHUNK * CH_PAD            # 131072 padded rows
NDENSE = NROW // 2048             # 64 dense chunks
DCH_PER = CH_PAD // 2048          # 16 dense chunks per node chunk
FSLICE = CH_PAD // NCORES         # 4096 rows per core per chunk (final phase)
F32 = mybir.dt.float32
I16 = mybir.dt.int16
I32 = mybir.dt.int32

MAX_WAITS = 1


def _split_sync_waits(nc):
    """walrus TPB codegen encodes at most one sync-wait per instruction;
    split longer wait lists into preceding same-engine nops."""
    n = 0
    for f in nc.m.functions:
        for blk in f.blocks:
            il = blk.instructions
            i = 0
            while i < len(il):
                ins = il[i]
                si = ins.sync_info
                if si is not None and si.on_wait and len(si.on_wait) > MAX_WAITS:
                    w = list(si.on_wait)
                    si.on_wait = w[-MAX_WAITS:]
                    ex = w[:-MAX_WAITS]
                    nops = []
                    for k in range(0, len(ex), MAX_WAITS):
                        p = mybir.InstNoOp(name=f"Wsplit-{n}-{k}")
                        p.engine = ins.engine
                        p.sync_info = mybir.SyncInfo(on_wait=ex[k:k + MAX_WAITS], on_update=[])
                        nops.append(p)
                    for j, p in enumerate(nops):
                        il.insert(i + j, p)
                    i += len(nops)
                    n += 1
                i += 1
    return n


def _shard_edges(src, dst):
    """Shard edges + self loops across cores; group per (src_chunk,
    dst_chunk); deal into unique-dst bins; pad bins to a structure common to
    all cores (the SPMD program is shared). Returns (plan, per_core) where
    plan = [(s, d, seg_lo, seg_hi, [(bin_lo, bin_hi), ...])] in slot units
    and per_core = list of dicts with int16 gidx/sidx flat slot arrays.
    """
    selfn = np.arange(N, dtype=np.int64)
    e_core = np.arange(E) % NCORES
    s_core = selfn % NCORES
    raw = []   # raw[c][seg] = (ss, dd, rank, ks)
    for c in range(NCORES):
        s = np.concatenate([src[e_core == c], selfn[s_core == c]])
        d = np.concatenate([dst[e_core == c], selfn[s_core == c]])
        seg_key = (s // CH_REAL) * NCHUNK + (d // CH_REAL)
        segs = {}
        for seg in range(NCHUNK * NCHUNK):
            m = seg_key == seg
            ss, dd = s[m], d[m]
            if len(dd):
                do = np.argsort(dd, kind="stable")
                ss, dd = ss[do], dd[do]
                grp = np.flatnonzero(np.r_[True, dd[1:] != dd[:-1]])
                rank = np.arange(len(dd)) - np.repeat(grp, np.diff(np.r_[grp, len(dd)]))
                ks = int(rank.max()) + 1
            else:
                rank, ks = np.zeros(0, np.int64), 0
            segs[seg] = (ss, dd, rank, ks)
        raw.append(segs)

    # common bin sizes (padded to 128 slots)
    plan = []
    slot = 0
    binsizes = {}
    for seg in range(NCHUNK * NCHUNK):
        ks = max(raw[c][seg][3] for c in range(NCORES))
        sizes = []
        for k in range(ks):
            mx = max(int((raw[c][seg][2] == k).sum()) for c in range(NCORES))
            sizes.append(-(-max(mx, 1) // 128) * 128)
        binsizes[seg] = sizes
        if ks:
            lo = slot
            spans = []
            for sz in sizes:
                spans.append((slot, slot + sz))
                slot += sz
            plan.append((seg // NCHUNK, seg % NCHUNK, lo, slot, spans))
    nslots = slot
    if nslots % 2048:
        pass  # slots are already multiples of 128; idx arrays use n/16 cols

    per_core = []
    rng = np.random.default_rng(1234)
    for c in range(NCORES):
        gi = np.zeros(nslots, np.int16)
        si = np.zeros(nslots, np.int16)
        pos = 0
        for (schunk, dchunk, lo, hi, spans) in plan:
            seg = schunk * NCHUNK + dchunk
            ss, dd, rank, _ = raw[c][seg]
            for k, (blo, bhi) in enumerate(spans):
                sz = bhi - blo
                m = rank == k
                bs = ss[m] - schunk * CH_REAL
                bd = dd[m] - dchunk * CH_REAL
                npad = sz - len(bs)
                assert npad >= 0
                if npad:
                    tp = CH_REAL + (np.arange(npad) % (CH_PAD - CH_REAL))
                    bs = np.concatenate([bs, np.zeros(npad, np.int64)])
                    bd = np.concatenate([bd, tp])
                gi[blo:bhi] = bs.astype(np.int16)
                si[blo:bhi] = bd.astype(np.int16)
        per_core.append(dict(gidx=gi, sidx=si))
    return plan, nslots, per_core


def _wrap16(a):
    """flat int16 index list (len % 16 == 0) -> [128, n/16] wrapped layout,
    replicated across the 8 GPSIMD core groups."""
    w = a.reshape(-1, 16).T.astype(np.int16)
    return np.ascontiguousarray(np.tile(w, (8, 1)))


def _build_program(plan, nslots, split=True):
    nc = bacc.Bacc("TRN2", target_bir_lowering=False, debug=False, num_devices=NCORES)
    AF = mybir.ActivationFunctionType

    t_ids16 = nc.dram_tensor("ids16", [128, NROW // 16], I16, kind="ExternalInput")
    t_batchf = nc.dram_tensor("batchf", [128, NCHUNK * FSLICE // 128], F32, kind="ExternalInput")
    t_gidx = nc.dram_tensor("gidx", [128, nslots // 16], I16, kind="ExternalInput")
    t_sidx = nc.dram_tensor("sidx", [128, nslots // 16], I16, kind="ExternalInput")
    t_fidx = nc.dram_tensor("fidx", [128, FSLICE // 16], I16, kind="ExternalInput")
    t_emb = nc.dram_tensor("emb", [NTYPES, EMB], F32, kind="ExternalInput")
    t_W1 = nc.dram_tensor("W1", [EMB, HID], F32, kind="ExternalInput")
    t_b1 = nc.dram_tensor("b1", [1, HID], F32, kind="ExternalInput")
    t_W2 = nc.dram_tensor("W2", [HID, HID], F32, kind="ExternalInput")
    t_b2 = nc.dram_tensor("b2", [1, HID], F32, kind="ExternalInput")
    t_Wc1 = nc.dram_tensor("Wc1", [HID, C1], F32, kind="ExternalInput")
    t_bc1 = nc.dram_tensor("bc1", [1, C1], F32, kind="ExternalInput")
    t_Wc2 = nc.dram_tensor("Wc2", [C1, 1], F32, kind="ExternalInput")
    t_bc2 = nc.dram_tensor("bc2", [1, 1], F32, kind="ExternalInput")
    t_iota = nc.dram_tensor("iota256", [128, G], F32, kind="ExternalInput")
    t_ident = nc.dram_tensor("ident128", [128, 128], F32, kind="ExternalInput")
    t_y = nc.dram_tensor("y", [G, 1], F32, kind="ExternalOutput")
    # zero-initialized by the runtime; reused as the t1 table after deg extraction
    t_deg = nc.dram_tensor("degtab", [NROW, EMB], F32, kind="ExternalOutput")

    t_u1 = nc.dram_tensor("u1tab", [NROW, EMB], F32)
    t_embw = nc.dram_tensor("embw", [256, EMB], F32)
    t_degc = nc.dram_tensor("degc", [NROW], F32)
    t_degr = nc.dram_tensor("degr", [NROW], F32, addr_space="Shared")
    acc1 = [nc.dram_tensor(f"acc1_{d}", [CH_PAD, EMB], F32) for d in range(NCHUNK)]
    acc2 = [nc.dram_tensor(f"acc2_{d}", [CH_PAD, EMB], F32) for d in range(NCHUNK)]
    acc1r = [nc.dram_tensor(f"acc1r_{d}", [CH_PAD, EMB], F32, addr_space="Shared")
             for d in range(NCHUNK)]
    acc2r = [nc.dram_tensor(f"acc2r_{d}", [CH_PAD, EMB], F32, addr_space="Shared")
             for d in range(NCHUNK)]
    t_pool = nc.dram_tensor("pooled", [G, 33], F32)
    t_poolr = nc.dram_tensor("pooledr", [G, 33], F32, addr_space="Shared")

    groups = [list(range(NCORES))]
    maxseg = max(hi - lo for (_, _, lo, hi, _) in plan)
    maxbin = max(bhi - blo for (*_, spans) in plan for (blo, bhi) in spans)

    with tile.TileContext(nc) as tc:
        with (
            tc.tile_pool(name="res", bufs=1) as res,
            tc.tile_pool(name="stage", bufs=2) as stage,
            tc.tile_pool(name="dense", bufs=3) as dense,
            tc.tile_pool(name="fine", bufs=3) as fine,
            tc.tile_pool(name="ps", bufs=2, space="PSUM") as psum,
            tc.tile_pool(name="poolacc", bufs=1, space="PSUM") as poolacc,
        ):
            # ---------- residents ----------
            ids16 = res.tile([128, NROW // 16], I16)
            nc.sync.dma_start(out=ids16[:], in_=t_ids16[:, :])
            gidx = res.tile([128, nslots // 16], I16)
            nc.sync.dma_start(out=gidx[:], in_=t_gidx[:, :])
            sidx = res.tile([128, nslots // 16], I16)
            nc.sync.dma_start(out=sidx[:], in_=t_sidx[:, :])
            fidx = res.tile([128, FSLICE // 16], I16)
            nc.sync.dma_start(out=fidx[:], in_=t_fidx[:, :])
            batchf = res.tile([128, NCHUNK * FSLICE // 128], F32)
            nc.sync.dma_start(out=batchf[:], in_=t_batchf[:, :])
            iota = res.tile([128, G], F32)
            nc.sync.dma_start(out=iota[:], in_=t_iota[:, :])
            ident = res.tile([128, 128], F32)
            nc.sync.dma_start(out=ident[:], in_=t_ident[:, :])
            onesP = res.tile([1, 128], F32)
            nc.vector.memset(onesP[:], 1.0)
            onestage = res.tile([128, maxbin // 128], F32)
            nc.vector.memset(onestage[:], 1.0)
            W1sb = res.tile([EMB, HID], F32)
            nc.sync.dma_start(out=W1sb[:], in_=t_W1[:, :])
            W2sb = res.tile([HID, HID], F32)
            nc.sync.dma_start(out=W2sb[:], in_=t_W2[:, :])
            Wc1sb = res.tile([HID, C1], F32)
            nc.sync.dma_start(out=Wc1sb[:], in_=t_Wc1[:, :])
            Wc2sb = res.tile([C1, 1], F32)
            nc.sync.dma_start(out=Wc2sb[:], in_=t_Wc2[:, :])

            def bcast_row(t_dram, w, nm):
                row = res.tile([1, w], F32, tag=f"row_{nm}")
                nc.sync.dma_start(out=row[:], in_=t_dram[:, :])
                p = psum.tile([128, w], F32, tag="pre")
                nc.tensor.matmul(out=p[:], lhsT=onesP[:], rhs=row[:], start=True, stop=True)
                out = res.tile([128, w], F32, tag=f"bc_{nm}")
                nc.vector.tensor_copy(out=out[:], in_=p[:])
                return out

            b1b = bcast_row(t_b1, HID, "b1")
            b2b = bcast_row(t_b2, HID, "b2")
            bc1b = bcast_row(t_bc1, C1, "bc1")
            bc2b = bcast_row(t_bc2, 1, "bc2")

            # ---------- embW1 = emb @ W1 ----------
            emb_lo = res.tile([128, EMB], F32)
            nc.sync.dma_start(out=emb_lo[:], in_=t_emb[0:128, :])
            ps1 = psum.tile([EMB, 128], F32, tag="pre")
            nc.tensor.transpose(out=ps1[:], in_=emb_lo[:], identity=ident[:])
            embT_lo = res.tile([EMB, 128], F32)
            nc.vector.tensor_copy(out=embT_lo[:], in_=ps1[:])
            emb_hi = res.tile([72, EMB], F32)
            nc.sync.dma_start(out=emb_hi[:], in_=t_emb[128:200, :])
            ps2 = psum.tile([EMB, 72], F32, tag="pre")
            nc.tensor.transpose(out=ps2[:], in_=emb_hi[:], identity=ident[0:72, 0:72])
            embT_hi = res.tile([EMB, 72], F32)
            nc.vector.tensor_copy(out=embT_hi[:], in_=ps2[:])
            ew_ps = psum.tile([128, HID], F32, tag="pre")
            nc.tensor.matmul(out=ew_ps[:], lhsT=embT_lo[:], rhs=W1sb[:], start=True, stop=True)
            ew_lo = res.tile([128, HID], F32)
            nc.vector.tensor_copy(out=ew_lo[:], in_=ew_ps[:])
            nc.sync.dma_start(out=t_embw[0:128, 0:HID], in_=ew_lo[:])
            ew_ps2 = psum.tile([72, HID], F32, tag="pre")
            nc.tensor.matmul(out=ew_ps2[:], lhsT=embT_hi[:], rhs=W1sb[:], start=True, stop=True)
            ew_hi = res.tile([72, HID], F32)
            nc.vector.tensor_copy(out=ew_hi[:], in_=ew_ps2[:])
            nc.sync.dma_start(out=t_embw[128:200, 0:HID], in_=ew_hi[:])

            # ---------- zero internal accumulators ----------
            zt = res.tile([128, 4096], F32)
            nc.vector.memset(zt[:], 0.0)
            for accs in (acc1, acc2):
                for a in accs:
                    av = a.ap().rearrange("(p q) e -> p (q e)", p=128)  # [128, 256*64]
                    for j in range(4):
                        nc.sync.dma_start(out=av[:, j * 4096:(j + 1) * 4096], in_=zt[:])

            # ---------- SWDGE serialization (descriptor-ring backpressure) ----------
            _sw = [None]

            def _chain(inst):
                if _sw[0] is not None:
                    add_dep_helper(inst.ins, _sw[0], reason="swdge chain")
                _sw[0] = inst.ins
                return inst

            SUB = 1024   # max indices per SWDGE op (ring capacity)

            # ---------- deg pass ----------
            for (schunk, dchunk, lo, hi, spans) in plan:
                dv = t_deg[dchunk * CH_PAD:(dchunk + 1) * CH_PAD, :]
                for (blo, bhi) in spans:
                    for q in range(blo, bhi, SUB):
                        qh = min(q + SUB, bhi)
                        nb = qh - q
                        _chain(nc.gpsimd.dma_scatter_add(
                            dv[:, 0:1], onestage[:, 0:nb // 128, None],
                            sidx[:, q // 16:qh // 16],
                            nb, nb, 1, elem_step=EMB, single_packet=False))

            # ---------- deg extraction + allreduce + c2 ----------
            for ch in range(NDENSE):
                dt_ = dense.tile([128, 16, EMB], F32, tag="dg_ld")
                nc.sync.dma_start(
                    out=dt_[:],
                    in_=t_deg[ch * 2048:(ch + 1) * 2048, :].rearrange("(p j) e -> p j e", p=128))
                dc = dense.tile([128, 16], F32, tag="dg_cp")
                nc.vector.tensor_copy(out=dc[:], in_=dt_[:, :, 0])
                nc.sync.dma_start(
                    out=t_degc[ch * 2048:(ch + 1) * 2048].rearrange("(p j) -> p j", p=128),
                    in_=dc[:])
            nc.gpsimd.collective_compute(
                "AllReduce", mybir.AluOpType.add, replica_groups=groups,
                ins=[t_degc.ap().opt()], outs=[t_degr.ap().opt()])

            c2 = res.tile([128, NDENSE, 16], F32)
            nc.sync.dma_start(
                out=c2[:], in_=t_degr.ap().rearrange("(g p j) -> p g j", p=128, j=16))
            nc.vector.tensor_scalar_max(out=c2[:], in0=c2[:], scalar1=1.0)
            nc.vector.reciprocal(out=c2[:], in_=c2[:])
            nc.scalar.activation(out=c2[:], in_=c2[:], func=AF.Sqrt)

            # ---------- t1 build (into degtab storage) ----------
            for ch in range(NDENSE):
                gt = dense.tile([128, 16, EMB], F32, tag="t1g")
                _chain(nc.gpsimd.dma_gather(
                    gt[:, 0:8, :], t_embw[:, :], ids16[:, ch * 128:ch * 128 + 64],
                    1024, 1024, EMB, single_packet=False))
                _chain(nc.gpsimd.dma_gather(
                    gt[:, 8:16, :], t_embw[:, :], ids16[:, ch * 128 + 64:(ch + 1) * 128],
                    1024, 1024, EMB, single_packet=False))
                cb = c2[:, ch, :, None].to_broadcast([128, 16, HID])
                t1c = dense.tile([128, 16, HID], F32, tag="t1c")
                nc.vector.tensor_mul(out=t1c[:], in0=gt[:, :, 0:HID], in1=cb)
                nc.sync.dma_start(
                    out=t_deg[ch * 2048:(ch + 1) * 2048, :]
                        .rearrange("(p j) e -> p j e", p=128)[:, :, 0:HID],
                    in_=t1c[:])

            # ---------- edge pass ----------
            def edge_pass(table, accs):
                for (schunk, dchunk, lo, hi, spans) in plan:
                    nseg = hi - lo
                    st = stage.tile([128, maxseg // 128, EMB], F32, tag="edgestage")
                    for q in range(0, nseg, SUB):
                        qh = min(q + SUB, nseg)
                        _chain(nc.gpsimd.dma_gather(
                            st[:, q // 128:qh // 128, :],
                            table[schunk * CH_PAD:(schunk + 1) * CH_PAD, :],
                            gidx[:, (lo + q) // 16:(lo + qh) // 16],
                            qh - q, qh - q, EMB, single_packet=False))
                    for (blo, bhi) in spans:
                        for q in range(blo, bhi, SUB):
                            qh = min(q + SUB, bhi)
                            nb = qh - q
                            _chain(nc.gpsimd.dma_scatter_add(
                                accs[dchunk][:, :],
                                st[:, (q - lo) // 128:(qh - lo) // 128, :],
                                sidx[:, q // 16:qh // 16],
                                nb, nb, EMB, single_packet=False))

            # ---------- L1 ----------
            edge_pass(t_deg, acc1)
            for d in range(NCHUNK):
                nc.gpsimd.collective_compute(
                    "AllReduce", mybir.AluOpType.add, replica_groups=groups,
                    ins=[acc1[d].ap().opt()], outs=[acc1r[d].ap().opt()])

            # ---------- interlayer: u1 = c * relu(c*acc1 + b1); col32 = 1 ----------
            b1v = b1b[:, None, :].to_broadcast([128, 16, HID])
            for ch in range(NDENSE):
                d = ch // DCH_PER
                off = (ch % DCH_PER) * 2048
                a = dense.tile([128, 16, EMB], F32, tag="il_ld")
                nc.sync.dma_start(
                    out=a[:],
                    in_=acc1r[d][off:off + 2048, :].rearrange("(p j) e -> p j e", p=128))
                cb = c2[:, ch, :, None].to_broadcast([128, 16, HID])
                y = dense.tile([128, 16, HID + 1], F32, tag="il_y")
                nc.vector.tensor_mul(out=y[:, :, 0:HID], in0=a[:, :, 0:HID], in1=cb)
                nc.vector.tensor_add(out=y[:, :, 0:HID], in0=y[:, :, 0:HID], in1=b1v)
                nc.scalar.activation(out=y[:, :, 0:HID], in_=y[:, :, 0:HID], func=AF.Relu)
                nc.vector.tensor_mul(out=y[:, :, 0:HID], in0=y[:, :, 0:HID], in1=cb)
                nc.vector.memset(y[:, :, HID:HID + 1], 1.0)
                nc.sync.dma_start(
                    out=t_u1[ch * 2048:(ch + 1) * 2048, :]
                        .rearrange("(p j) e -> p j e", p=128)[:, :, 0:HID + 1],
                    in_=y[:])

            # ---------- L2 ----------
            edge_pass(t_u1, acc2)
            for d in range(NCHUNK):
                nc.gpsimd.collective_compute(
                    "AllReduce", mybir.AluOpType.add, replica_groups=groups,
                    ins=[acc2[d].ap().opt()], outs=[acc2r[d].ap().opt()])

            # ---------- final: per-core slice via fidx gathers ----------
            pool_lo = poolacc.tile([128, 33], F32)
            pool_hi = poolacc.tile([128, 33], F32)
            NS = FSLICE // 128   # 32 subtiles per chunk
            first = True
            for d in range(NCHUNK):
                fs = fine.tile([128, NS, EMB], F32, tag="fstage")
                for q in range(0, FSLICE, SUB):
                    qh = q + SUB
                    _chain(nc.gpsimd.dma_gather(
                        fs[:, q // 128:qh // 128, :], acc2r[d][:, :],
                        fidx[:, q // 16:qh // 16],
                        SUB, SUB, EMB, single_packet=False))
                # c for these rows from deg in col 32
                cch = fine.tile([128, NS], F32, tag="fc")
                nc.vector.tensor_copy(out=cch[:], in_=fs[:, :, HID])
                nc.vector.tensor_scalar_max(out=cch[:], in0=cch[:], scalar1=1.0)
                nc.vector.reciprocal(out=cch[:], in_=cch[:])
                nc.scalar.activation(out=cch[:], in_=cch[:], func=AF.Sqrt)
                for s in range(NS):
                    y2T_ps = psum.tile([HID, 128], F32, tag="fin_t")
                    nc.tensor.transpose(out=y2T_ps[:], in_=fs[:, s, 0:HID], identity=ident[:])
                    y2T = fine.tile([HID, 128], F32, tag="fin_ts")
                    nc.vector.tensor_copy(out=y2T[:], in_=y2T_ps[:])
                    xw_ps = psum.tile([128, HID], F32, tag="fin_mm")
                    nc.tensor.matmul(out=xw_ps[:], lhsT=y2T[:], rhs=W2sb[:], start=True, stop=True)
                    h2e = fine.tile([128, 33], F32, tag="fin_h2")
                    nc.vector.tensor_scalar_mul(out=h2e[:, 0:HID], in0=xw_ps[:], scalar1=cch[:, s:s + 1])
                    nc.vector.tensor_add(out=h2e[:, 0:HID], in0=h2e[:, 0:HID], in1=b2b[:])
                    nc.scalar.activation(out=h2e[:, 0:HID], in_=h2e[:, 0:HID], func=AF.Relu)
                    nc.vector.memset(h2e[:, HID:HID + 1], 1.0)
                    oh = fine.tile([128, G], F32, tag="fin_oh")
                    nc.vector.tensor_tensor(
                        out=oh[:], in0=batchf[:, d * NS + s, None].to_broadcast([128, G]),
                        in1=iota[:], op=mybir.AluOpType.is_equal)
                    nc.tensor.matmul(out=pool_lo[:], lhsT=oh[:, 0:128], rhs=h2e[:],
                                     start=first, stop=(d == NCHUNK - 1 and s == NS - 1))
                    nc.tensor.matmul(out=pool_hi[:], lhsT=oh[:, 128:G], rhs=h2e[:],
                                     start=first, stop=(d == NCHUNK - 1 and s == NS - 1))
                    first = False

            pl = res.tile([128, 33], F32, tag="pl")
            ph = res.tile([128, 33], F32, tag="ph")
            nc.vector.tensor_copy(out=pl[:], in_=pool_lo[:])
            nc.vector.tensor_copy(out=ph[:], in_=pool_hi[:])
            nc.sync.dma_start(out=t_pool[0:128, :], in_=pl[:])
            nc.sync.dma_start(out=t_pool[128:G, :], in_=ph[:])
            nc.gpsimd.collective_compute(
                "AllReduce", mybir.AluOpType.add, replica_groups=groups,
                ins=[t_pool.ap().opt()], outs=[t_poolr.ap().opt()])

            # ---------- mean + MLP head (every core computes the same y) ----------
            for half in range(2):
                pr = res.tile([128, 33], F32, tag=f"pr{half}")
                nc.sync.dma_start(out=pr[:], in_=t_poolr[half * 128:(half + 1) * 128, :])
                cnt = res.tile([128, 1], F32, tag=f"cnt{half}")
                nc.vector.tensor_scalar_max(out=cnt[:], in0=pr[:, 32:33], scalar1=1.0)
                nc.vector.reciprocal(out=cnt[:], in_=cnt[:])
                mean = res.tile([128, HID], F32, tag=f"mean{half}")
                nc.vector.tensor_scalar_mul(out=mean[:], in0=pr[:, 0:HID], scalar1=cnt[:])
                mT_ps = psum.tile([HID, 128], F32, tag="pre")
                nc.tensor.transpose(out=mT_ps[:], in_=mean[:], identity=ident[:])
                mT = res.tile([HID, 128], F32, tag=f"mT{half}")
                nc.vector.tensor_copy(out=mT[:], in_=mT_ps[:])
                hc_ps = psum.tile([128, C1], F32, tag="pre")
                nc.tensor.matmul(out=hc_ps[:], lhsT=mT[:], rhs=Wc1sb[:], start=True, stop=True)
                hc = res.tile([128, C1], F32, tag=f"hc{half}")
                nc.vector.tensor_add(out=hc[:], in0=hc_ps[:], in1=bc1b[:])
                nc.scalar.activation(out=hc[:], in_=hc[:], func=AF.Relu)
                hT_ps = psum.tile([C1, 128], F32, tag="pre")
                nc.tensor.transpose(out=hT_ps[:], in_=hc[:], identity=ident[:])
                hT = res.tile([C1, 128], F32, tag=f"hT{half}")
                nc.vector.tensor_copy(out=hT[:], in_=hT_ps[:])
                o_ps = psum.tile([128, 1], F32, tag="pre")
                nc.tensor.matmul(out=o_ps[:], lhsT=hT[:], rhs=Wc2sb[:], start=True, stop=True)
                ob = res.tile([128, 1], F32, tag=f"ob{half}")
                nc.vector.tensor_add(out=ob[:], in0=o_ps[:], in1=bc2b[:])
                nc.scalar.activation(out=ob[:], in_=ob[:], func=AF.Sigmoid)
                nc.sync.dma_start(out=t_y[half * 128:(half + 1) * 128, :], in_=ob[:])

    nc.compile()
    if split:
        _split_sync_waits(nc)
    return nc


_PROG_CACHE = {}


def kernel(**inputs):
    x = np.asarray(inputs["x"]).astype(np.int64).reshape(-1)
    ei = np.asarray(inputs["edge_index"]).astype(np.int64)
    batch = np.asarray(inputs["batch"]).astype(np.int64).reshape(-1)
    emb = np.asarray(inputs["emb"], np.float32)
    W1 = np.asarray(inputs["W1"], np.float32)
    b1 = np.asarray(inputs["b1"], np.float32).reshape(1, -1)
    W2 = np.asarray(inputs["W2"], np.float32)
    b2 = np.asarray(inputs["b2"], np.float32).reshape(1, -1)
    Wc1 = np.asarray(inputs["Wc1"], np.float32)
    bc1 = np.asarray(inputs["bc1"], np.float32).reshape(1, -1)
    Wc2 = np.asarray(inputs["Wc2"], np.float32)
    bc2 = np.asarray(inputs["bc2"], np.float32).reshape(1, -1)

    plan, nslots, per_core = _shard_edges(ei[0], ei[1])

    key = (nslots, tuple((s, d, lo, hi, tuple(sp)) for (s, d, lo, hi, sp) in plan))
    if key not in _PROG_CACHE:
        _PROG_CACHE[key] = _build_program(plan, nslots)
    nc = _PROG_CACHE[key]

    # node-id table in padded row space (row = (n//CH_REAL)*CH_PAD + n%CH_REAL)
    ids_pad = np.zeros(NROW, np.int64)
    rows = (np.arange(N) // CH_REAL) * CH_PAD + (np.arange(N) % CH_REAL)
    ids_pad[rows] = x
    ids16 = _wrap16(ids_pad.astype(np.int16))

    # batch values for each core's final-phase rows: core c, chunk d,
    # subtile s, partition p -> chunk-local row FSLICE*c + s*128 + p
    iota256 = np.tile(np.arange(G, dtype=np.float32), (128, 1))
    ident128 = np.eye(128, dtype=np.float32)

    loc = np.arange(FSLICE)
    in_maps = []
    for c in range(NCORES):
        fl = FSLICE * c + loc          # chunk-local rows this core handles
        fidx = _wrap16(fl.astype(np.int16))
        bvals = np.full((NCHUNK, FSLICE), -1.0, np.float32)
        for d in range(NCHUNK):
            gl = fl.copy()
            real = gl < CH_REAL
            n_global = d * CH_REAL + gl
            ok = real & (n_global < N)
            bvals[d, ok] = batch[n_global[ok]]
        # [128, NCHUNK*FSLICE/128] with col d*NS + s at partition p = row s*128+p
        barr = bvals.reshape(NCHUNK, FSLICE // 128, 128).transpose(2, 0, 1).reshape(
            128, NCHUNK * (FSLICE // 128))
        in_maps.append(dict(
            ids16=ids16,
            batchf=np.ascontiguousarray(barr),
            gidx=_wrap16(per_core[c]["gidx"]),
            sidx=_wrap16(per_core[c]["sidx"]),
            fidx=fidx,
            emb=emb, W1=W1, b1=b1, W2=W2, b2=b2,
            Wc1=Wc1, bc1=bc1, Wc2=Wc2, bc2=bc2,
            iota256=iota256, ident128=ident128,
        ))

    res = run_bass_kernel_spmd(nc, in_maps, core_ids=list(range(NCORES)))
    return res.results[0]["y"].astype(np.float32)



# revision 3
# speedup vs baseline: 1050.2448x; 1050.2448x over previous
"""Trainium2 Bass kernel for a 2-layer GCN + global mean pool + MLP head.

Distribution (8 NeuronCores): edge-parallel. Edges (plus one self-loop per
node) are sharded across cores as part of input distribution; each core
gathers node-table rows by src (dma_gather) and scatter-adds them by dst
(dma_scatter_add with SDMA CCE f32 add) into per-core partial accumulators;
node-boundary partial sums are combined with AllReduce. Small parameters are
replicated.

Math: with c = rsqrt(deg) (deg counts in-edges incl. the self loop), each
GCN layer is  h' = relu(c * (sum_{u->v} t[u]) + b)  with  t = c * (h @ W).
The layer-2 weight multiply commutes with the edge sum, so the second edge
pass scatters u1 = c * h1 rows and W2 is applied after the reduce. Column 32
of the u1 rows carries the constant 1, so acc2[:,32] reproduces deg and the
final phase is self-contained per gathered row.

Race-freedom: duplicate scatter destinations within one dma_scatter_add and
across concurrently-running ones are not accumulated correctly by the DMA
engines, so the host deals each (src-chunk, dst-chunk) edge segment into
bins with unique dst (rank-within-dst dealing), and all scatter instructions
that target the same dst-chunk accumulator are chained with explicit deps.
"""

import numpy as np

import concourse.bacc as bacc
import concourse.mybir as mybir
import concourse.tile as tile
from concourse.bass_utils import run_bass_kernel_spmd
from bass_rust import add_dep_helper

# ---- problem geometry (hardcoded per task contract) ----
N = 100000
E = 1000000
G = 256
NTYPES = 200
EMB = 64            # embedding dim; also the 256B table row width (64 f32)
HID = 32
C1 = 16
NCORES = 8

CH_REAL = 25600     # real node rows per chunk (int16-addressable)
CH_PAD = 32768      # chunk stride (16 * 2048)
NCHUNK = 4
NROW = NCHUNK * CH_PAD            # 131072 padded rows
NDENSE = NROW // 2048             # 64 dense chunks
DCH_PER = CH_PAD // 2048          # 16 dense chunks per node chunk
FSLICE = CH_PAD // NCORES         # 4096 rows per core per chunk (final phase)
F32 = mybir.dt.float32
I16 = mybir.dt.int16
I32 = mybir.dt.int32

MAX_WAITS = 1


def _split_sync_waits(nc):
    """walrus TPB codegen encodes at most one sync-wait per instruction;
    split longer wait lists into preceding same-engine nops."""
    n = 0
    for f in nc.m.functions:
        for blk in f.blocks:
            il = blk.instructions
            i = 0
            while i < len(il):
                ins = il[i]
                si = ins.sync_info
                if si is not None and si.on_wait and len(si.on_wait) > MAX_WAITS:
                    w = list(si.on_wait)
                    si.on_wait = w[-MAX_WAITS:]
                    ex = w[:-MAX_WAITS]
                    nops = []
                    for k in range(0, len(ex), MAX_WAITS):
                        p = mybir.InstNoOp(name=f"Wsplit-{n}-{k}")
                        p.engine = ins.engine
                        p.sync_info = mybir.SyncInfo(on_wait=ex[k:k + MAX_WAITS], on_update=[])
                        nops.append(p)
                    for j, p in enumerate(nops):
                        il.insert(i + j, p)
                    i += len(nops)
                    n += 1
                i += 1
    return n


def _shard_edges(src, dst):
    """Shard edges + self loops across cores; group per (src_chunk,
    dst_chunk); deal into unique-dst bins; pad bins to a structure common to
    all cores (the SPMD program is shared). Returns (plan, per_core) where
    plan = [(s, d, seg_lo, seg_hi, [(bin_lo, bin_hi), ...])] in slot units
    and per_core = list of dicts with int16 gidx/sidx flat slot arrays.
    """
    selfn = np.arange(N, dtype=np.int64)
    e_core = np.arange(E) % NCORES
    s_core = selfn % NCORES
    raw = []   # raw[c][seg] = (ss, dd, rank, ks)
    for c in range(NCORES):
        s = np.concatenate([src[e_core == c], selfn[s_core == c]])
        d = np.concatenate([dst[e_core == c], selfn[s_core == c]])
        seg_key = (s // CH_REAL) * NCHUNK + (d // CH_REAL)
        segs = {}
        for seg in range(NCHUNK * NCHUNK):
            m = seg_key == seg
            ss, dd = s[m], d[m]
            if len(dd):
                do = np.argsort(dd, kind="stable")
                ss, dd = ss[do], dd[do]
                grp = np.flatnonzero(np.r_[True, dd[1:] != dd[:-1]])
                rank = np.arange(len(dd)) - np.repeat(grp, np.diff(np.r_[grp, len(dd)]))
                ks = int(rank.max()) + 1
            else:
                rank, ks = np.zeros(0, np.int64), 0
            segs[seg] = (ss, dd, rank, ks)
        raw.append(segs)

    # common bin sizes (padded to 128 slots)
    plan = []
    slot = 0
    binsizes = {}
    for seg in range(NCHUNK * NCHUNK):
        ks = max(raw[c][seg][3] for c in range(NCORES))
        sizes = []
        for k in range(ks):
            mx = max(int((raw[c][seg][2] == k).sum()) for c in range(NCORES))
            sizes.append(-(-max(mx, 1) // 128) * 128)
        binsizes[seg] = sizes
        if ks:
            lo = slot
            spans = []
            for sz in sizes:
                spans.append((slot, slot + sz))
                slot += sz
            plan.append((seg // NCHUNK, seg % NCHUNK, lo, slot, spans))
    nslots = slot
    if nslots % 2048:
        pass  # slots are already multiples of 128; idx arrays use n/16 cols

    per_core = []
    rng = np.random.default_rng(1234)
    for c in range(NCORES):
        gi = np.zeros(nslots, np.int16)
        si = np.zeros(nslots, np.int16)
        pos = 0
        for (schunk, dchunk, lo, hi, spans) in plan:
            seg = schunk * NCHUNK + dchunk
            ss, dd, rank, _ = raw[c][seg]
            for k, (blo, bhi) in enumerate(spans):
                sz = bhi - blo
                m = rank == k
                bs = ss[m] - schunk * CH_REAL
                bd = dd[m] - dchunk * CH_REAL
                npad = sz - len(bs)
                assert npad >= 0
                if npad:
                    tp = CH_REAL + (np.arange(npad) % (CH_PAD - CH_REAL))
                    bs = np.concatenate([bs, np.zeros(npad, np.int64)])
                    bd = np.concatenate([bd, tp])
                gi[blo:bhi] = bs.astype(np.int16)
                si[blo:bhi] = bd.astype(np.int16)
        per_core.append(dict(gidx=gi, sidx=si))
    return plan, nslots, per_core


def _wrap16(a):
    """flat int16 index list (len % 16 == 0) -> [128, n/16] wrapped layout,
    replicated across the 8 GPSIMD core groups."""
    w = a.reshape(-1, 16).T.astype(np.int16)
    return np.ascontiguousarray(np.tile(w, (8, 1)))


def _build_program(plan, nslots, split=True):
    nc = bacc.Bacc("TRN2", target_bir_lowering=False, debug=False, num_devices=NCORES)
    AF = mybir.ActivationFunctionType

    t_ids16 = nc.dram_tensor("ids16", [128, NROW // 16], I16, kind="ExternalInput")
    t_batchf = nc.dram_tensor("batchf", [128, NCHUNK * FSLICE // 128], F32, kind="ExternalInput")
    t_gidx = nc.dram_tensor("gidx", [128, nslots // 16], I16, kind="ExternalInput")
    t_sidx = nc.dram_tensor("sidx", [128, nslots // 16], I16, kind="ExternalInput")
    t_fidx = nc.dram_tensor("fidx", [128, FSLICE // 16], I16, kind="ExternalInput")
    t_emb = nc.dram_tensor("emb", [NTYPES, EMB], F32, kind="ExternalInput")
    t_W1 = nc.dram_tensor("W1", [EMB, HID], F32, kind="ExternalInput")
    t_b1 = nc.dram_tensor("b1", [1, HID], F32, kind="ExternalInput")
    t_W2 = nc.dram_tensor("W2", [HID, HID], F32, kind="ExternalInput")
    t_b2 = nc.dram_tensor("b2", [1, HID], F32, kind="ExternalInput")
    t_Wc1 = nc.dram_tensor("Wc1", [HID, C1], F32, kind="ExternalInput")
    t_bc1 = nc.dram_tensor("bc1", [1, C1], F32, kind="ExternalInput")
    t_Wc2 = nc.dram_tensor("Wc2", [C1, 1], F32, kind="ExternalInput")
    t_bc2 = nc.dram_tensor("bc2", [1, 1], F32, kind="ExternalInput")
    t_iota = nc.dram_tensor("iota256", [128, G], F32, kind="ExternalInput")
    t_ident = nc.dram_tensor("ident128", [128, 128], F32, kind="ExternalInput")
    t_y = nc.dram_tensor("y", [G, 1], F32, kind="ExternalOutput")
    # zero-initialized by the runtime; reused as the t1 table after deg extraction
    t_deg = nc.dram_tensor("degtab", [NROW, EMB], F32, kind="ExternalOutput")

    t_u1 = nc.dram_tensor("u1tab", [NROW, EMB], F32)
    t_embw = nc.dram_tensor("embw", [256, EMB], F32)
    t_degc = nc.dram_tensor("degc", [NROW], F32)
    t_degr = nc.dram_tensor("degr", [NROW], F32, addr_space="Shared")
    acc1 = [nc.dram_tensor(f"acc1_{d}", [CH_PAD, EMB], F32) for d in range(NCHUNK)]
    acc2 = [nc.dram_tensor(f"acc2_{d}", [CH_PAD, EMB], F32) for d in range(NCHUNK)]
    acc1r = [nc.dram_tensor(f"acc1r_{d}", [CH_PAD, EMB], F32, addr_space="Shared")
             for d in range(NCHUNK)]
    acc2r = [nc.dram_tensor(f"acc2r_{d}", [CH_PAD, EMB], F32, addr_space="Shared")
             for d in range(NCHUNK)]
    t_pool = nc.dram_tensor("pooled", [G, 33], F32)
    t_poolr = nc.dram_tensor("pooledr", [G, 33], F32, addr_space="Shared")

    groups = [list(range(NCORES))]
    maxseg = max(hi - lo for (_, _, lo, hi, _) in plan)
    maxbin = max(bhi - blo for (*_, spans) in plan for (blo, bhi) in spans)

    with tile.TileContext(nc) as tc:
        with (
            tc.tile_pool(name="res", bufs=1) as res,
            tc.tile_pool(name="stage", bufs=2) as stage,
            tc.tile_pool(name="dense", bufs=3) as dense,
            tc.tile_pool(name="fine", bufs=3) as fine,
            tc.tile_pool(name="ps", bufs=2, space="PSUM") as psum,
            tc.tile_pool(name="poolacc", bufs=1, space="PSUM") as poolacc,
        ):
            # ---------- residents ----------
            ids16 = res.tile([128, NROW // 16], I16)
            nc.sync.dma_start(out=ids16[:], in_=t_ids16[:, :])
            gidx = res.tile([128, nslots // 16], I16)
            nc.sync.dma_start(out=gidx[:], in_=t_gidx[:, :])
            sidx = res.tile([128, nslots // 16], I16)
            nc.sync.dma_start(out=sidx[:], in_=t_sidx[:, :])
            fidx = res.tile([128, FSLICE // 16], I16)
            nc.sync.dma_start(out=fidx[:], in_=t_fidx[:, :])
            batchf = res.tile([128, NCHUNK * FSLICE // 128], F32)
            nc.sync.dma_start(out=batchf[:], in_=t_batchf[:, :])
            iota = res.tile([128, G], F32)
            nc.sync.dma_start(out=iota[:], in_=t_iota[:, :])
            ident = res.tile([128, 128], F32)
            nc.sync.dma_start(out=ident[:], in_=t_ident[:, :])
            onesP = res.tile([1, 128], F32)
            nc.vector.memset(onesP[:], 1.0)
            onestage = res.tile([128, maxbin // 128], F32)
            nc.vector.memset(onestage[:], 1.0)
            W1sb = res.tile([EMB, HID], F32)
            nc.sync.dma_start(out=W1sb[:], in_=t_W1[:, :])
            W2sb = res.tile([HID, HID], F32)
            nc.sync.dma_start(out=W2sb[:], in_=t_W2[:, :])
            Wc1sb = res.tile([HID, C1], F32)
            nc.sync.dma_start(out=Wc1sb[:], in_=t_Wc1[:, :])
            Wc2sb = res.tile([C1, 1], F32)
            nc.sync.dma_start(out=Wc2sb[:], in_=t_Wc2[:, :])

            def bcast_row(t_dram, w, nm):
                row = res.tile([1, w], F32, tag=f"row_{nm}")
                nc.sync.dma_start(out=row[:], in_=t_dram[:, :])
                p = psum.tile([128, w], F32, tag="pre")
                nc.tensor.matmul(out=p[:], lhsT=onesP[:], rhs=row[:], start=True, stop=True)
                out = res.tile([128, w], F32, tag=f"bc_{nm}")
                nc.vector.tensor_copy(out=out[:], in_=p[:])
                return out

            b1b = bcast_row(t_b1, HID, "b1")
            b2b = bcast_row(t_b2, HID, "b2")
            bc1b = bcast_row(t_bc1, C1, "bc1")
            bc2b = bcast_row(t_bc2, 1, "bc2")

            # ---------- embW1 = emb @ W1 ----------
            emb_lo = res.tile([128, EMB], F32)
            nc.sync.dma_start(out=emb_lo[:], in_=t_emb[0:128, :])
            ps1 = psum.tile([EMB, 128], F32, tag="pre")
            nc.tensor.transpose(out=ps1[:], in_=emb_lo[:], identity=ident[:])
            embT_lo = res.tile([EMB, 128], F32)
            nc.vector.tensor_copy(out=embT_lo[:], in_=ps1[:])
            emb_hi = res.tile([72, EMB], F32)
            nc.sync.dma_start(out=emb_hi[:], in_=t_emb[128:200, :])
            ps2 = psum.tile([EMB, 72], F32, tag="pre")
            nc.tensor.transpose(out=ps2[:], in_=emb_hi[:], identity=ident[0:72, 0:72])
            embT_hi = res.tile([EMB, 72], F32)
            nc.vector.tensor_copy(out=embT_hi[:], in_=ps2[:])
            ew_ps = psum.tile([128, HID], F32, tag="pre")
            nc.tensor.matmul(out=ew_ps[:], lhsT=embT_lo[:], rhs=W1sb[:], start=True, stop=True)
            ew_lo = res.tile([128, HID], F32)
            nc.vector.tensor_copy(out=ew_lo[:], in_=ew_ps[:])
            nc.sync.dma_start(out=t_embw[0:128, 0:HID], in_=ew_lo[:])
            ew_ps2 = psum.tile([72, HID], F32, tag="pre")
            nc.tensor.matmul(out=ew_ps2[:], lhsT=embT_hi[:], rhs=W1sb[:], start=True, stop=True)
            ew_hi = res.tile([72, HID], F32)
            nc.vector.tensor_copy(out=ew_hi[:], in_=ew_ps2[:])
            nc.sync.dma_start(out=t_embw[128:200, 0:HID], in_=ew_hi[:])

            # ---------- zero internal accumulators ----------
            zt = res.tile([128, 4096], F32)
            nc.vector.memset(zt[:], 0.0)
            for accs in (acc1, acc2):
                for a in accs:
                    av = a.ap().rearrange("(p q) e -> p (q e)", p=128)  # [128, 256*64]
                    for j in range(4):
                        nc.sync.dma_start(out=av[:, j * 4096:(j + 1) * 4096], in_=zt[:])

            # ---------- SWDGE serialization (descriptor-ring backpressure) ----------
            _sw = [None]

            def _chain(inst):
                if _sw[0] is not None:
                    add_dep_helper(inst.ins, _sw[0], reason="swdge chain")
                _sw[0] = inst.ins
                return inst

            SUB = 1024   # max indices per SWDGE op (ring capacity)

            # ---------- deg pass ----------
            for (schunk, dchunk, lo, hi, spans) in plan:
                dv = t_deg[dchunk * CH_PAD:(dchunk + 1) * CH_PAD, :]
                for (blo, bhi) in spans:
                    for q in range(blo, bhi, SUB):
                        qh = min(q + SUB, bhi)
                        nb = qh - q
                        _chain(nc.gpsimd.dma_scatter_add(
                            dv[:, 0:1], onestage[:, 0:nb // 128, None],
                            sidx[:, q // 16:qh // 16],
                            nb, nb, 1, elem_step=EMB, single_packet=False))

            # ---------- deg extraction + allreduce + c2 ----------
            for ch in range(NDENSE):
                dt_ = dense.tile([128, 16, EMB], F32, tag="dg_ld")
                nc.sync.dma_start(
                    out=dt_[:],
                    in_=t_deg[ch * 2048:(ch + 1) * 2048, :].rearrange("(p j) e -> p j e", p=128))
                dc = dense.tile([128, 16], F32, tag="dg_cp")
                nc.vector.tensor_copy(out=dc[:], in_=dt_[:, :, 0])
                nc.sync.dma_start(
                    out=t_degc[ch * 2048:(ch + 1) * 2048].rearrange("(p j) -> p j", p=128),
                    in_=dc[:])
            nc.gpsimd.collective_compute(
                "AllReduce", mybir.AluOpType.add, replica_groups=groups,
                ins=[t_degc.ap().opt()], outs=[t_degr.ap().opt()])

            c2 = res.tile([128, NDENSE, 16], F32)
            nc.sync.dma_start(
                out=c2[:], in_=t_degr.ap().rearrange("(g p j) -> p g j", p=128, j=16))
            nc.vector.tensor_scalar_max(out=c2[:], in0=c2[:], scalar1=1.0)
            nc.vector.reciprocal(out=c2[:], in_=c2[:])
            nc.scalar.activation(out=c2[:], in_=c2[:], func=AF.Sqrt)

            # ---------- t1 build (into degtab storage) ----------
            for ch in range(NDENSE):
                gt = dense.tile([128, 16, EMB], F32, tag="t1g")
                _chain(nc.gpsimd.dma_gather(
                    gt[:, 0:8, :], t_embw[:, :], ids16[:, ch * 128:ch * 128 + 64],
                    1024, 1024, EMB, single_packet=False))
                _chain(nc.gpsimd.dma_gather(
                    gt[:, 8:16, :], t_embw[:, :], ids16[:, ch * 128 + 64:(ch + 1) * 128],
                    1024, 1024, EMB, single_packet=False))
                cb = c2[:, ch, :, None].to_broadcast([128, 16, HID])
                t1c = dense.tile([128, 16, HID], F32, tag="t1c")
                nc.vector.tensor_mul(out=t1c[:], in0=gt[:, :, 0:HID], in1=cb)
                nc.sync.dma_start(
                    out=t_deg[ch * 2048:(ch + 1) * 2048, :]
                        .rearrange("(p j) e -> p j e", p=128)[:, :, 0:HID],
                    in_=t1c[:])

            # ---------- edge pass ----------
            def edge_pass(table, accs):
                for (schunk, dchunk, lo, hi, spans) in plan:
                    nseg = hi - lo
                    st = stage.tile([128, maxseg // 128, EMB], F32, tag="edgestage")
                    for q in range(0, nseg, SUB):
                        qh = min(q + SUB, nseg)
                        _chain(nc.gpsimd.dma_gather(
                            st[:, q // 128:qh // 128, :],
                            table[schunk * CH_PAD:(schunk + 1) * CH_PAD, :],
                            gidx[:, (lo + q) // 16:(lo + qh) // 16],
                            qh - q, qh - q, EMB, single_packet=False))
                    for (blo, bhi) in spans:
                        for q in range(blo, bhi, SUB):
                            qh = min(q + SUB, bhi)
                            nb = qh - q
                            _chain(nc.gpsimd.dma_scatter_add(
                                accs[dchunk][:, :],
                                st[:, (q - lo) // 128:(qh - lo) // 128, :],
                                sidx[:, q // 16:qh // 16],
                                nb, nb, EMB, single_packet=False))

            # ---------- L1 ----------
            edge_pass(t_deg, acc1)
            for d in range(NCHUNK):
                nc.gpsimd.collective_compute(
                    "AllReduce", mybir.AluOpType.add, replica_groups=groups,
                    ins=[acc1[d].ap().opt()], outs=[acc1r[d].ap().opt()])

            # ---------- interlayer: u1 = c * relu(c*acc1 + b1); col32 = 1 ----------
            b1v = b1b[:, None, :].to_broadcast([128, 16, HID])
            for ch in range(NDENSE):
                d = ch // DCH_PER
                off = (ch % DCH_PER) * 2048
                a = dense.tile([128, 16, EMB], F32, tag="il_ld")
                nc.sync.dma_start(
                    out=a[:],
                    in_=acc1r[d][off:off + 2048, :].rearrange("(p j) e -> p j e", p=128))
                cb = c2[:, ch, :, None].to_broadcast([128, 16, HID])
                y = dense.tile([128, 16, HID + 1], F32, tag="il_y")
                nc.vector.tensor_mul(out=y[:, :, 0:HID], in0=a[:, :, 0:HID], in1=cb)
                nc.vector.tensor_add(out=y[:, :, 0:HID], in0=y[:, :, 0:HID], in1=b1v)
                nc.scalar.activation(out=y[:, :, 0:HID], in_=y[:, :, 0:HID], func=AF.Relu)
                nc.vector.tensor_mul(out=y[:, :, 0:HID], in0=y[:, :, 0:HID], in1=cb)
                nc.vector.memset(y[:, :, HID:HID + 1], 1.0)
                nc.sync.dma_start(
                    out=t_u1[ch * 2048:(ch + 1) * 2048, :]
                        .rearrange("(p j) e -> p j e", p=128)[:, :, 0:HID + 1],
                    in_=y[:])

            # ---------- L2 ----------
            edge_pass(t_u1, acc2)
            for d in range(NCHUNK):
                nc.gpsimd.collective_compute(
                    "AllReduce", mybir.AluOpType.add, replica_groups=groups,
                    ins=[acc2[d].ap().opt()], outs=[acc2r[d].ap().opt()])

            # ---------- final: per-core slice via fidx gathers ----------
            pool_lo = poolacc.tile([128, 33], F32)
            pool_hi = poolacc.tile([128, 33], F32)
            NS = FSLICE // 128   # 32 subtiles per chunk
            first = True
            for d in range(NCHUNK):
                fs = fine.tile([128, NS, EMB], F32, tag="fstage")
                for q in range(0, FSLICE, SUB):
                    qh = q + SUB
                    _chain(nc.gpsimd.dma_gather(
                        fs[:, q // 128:qh // 128, :], acc2r[d][:, :],
                        fidx[:, q // 16:qh // 16],
                        SUB, SUB, EMB, single_packet=False))
                # c for these rows from deg in col 32
                cch = fine.tile([128, NS], F32, tag="fc")
                nc.vector.tensor_copy(out=cch[:], in_=fs[:, :, HID])
                nc.vector.tensor_scalar_max(out=cch[:], in0=cch[:], scalar1=1.0)
                nc.vector.reciprocal(out=cch[:], in_=cch[:])
                nc.scalar.activation(out=cch[:], in_=cch[:], func=AF.Sqrt)
                for s in range(NS):
                    y2T_ps = psum.tile([HID, 128], F32, tag="fin_t")
                    nc.tensor.transpose(out=y2T_ps[:], in_=fs[:, s, 0:HID], identity=ident[:])
                    y2T = fine.tile([HID, 128], F32, tag="fin_ts")
                    nc.vector.tensor_copy(out=y2T[:], in_=y2T_ps[:])
                    xw_ps = psum.tile([128, HID], F32, tag="fin_mm")
                    nc.tensor.matmul(out=xw_ps[:], lhsT=y2T[:], rhs=W2sb[:], start=True, stop=True)
                    h2e = fine.tile([128, 33], F32, tag="fin_h2")
                    nc.vector.tensor_scalar_mul(out=h2e[:, 0:HID], in0=xw_ps[:], scalar1=cch[:, s:s + 1])
                    nc.vector.tensor_add(out=h2e[:, 0:HID], in0=h2e[:, 0:HID], in1=b2b[:])
                    nc.scalar.activation(out=h2e[:, 0:HID], in_=h2e[:, 0:HID], func=AF.Relu)
                    nc.vector.memset(h2e[:, HID:HID + 1], 1.0)
                    oh = fine.tile([128, G], F32, tag="fin_oh")
                    nc.vector.tensor_tensor(
                        out=oh[:], in0=batchf[:, d * NS + s, None].to_broadcast([128, G]),
                        in1=iota[:], op=mybir.AluOpType.is_equal)
                    nc.tensor.matmul(out=pool_lo[:], lhsT=oh[:, 0:128], rhs=h2e[:],
                                     start=first, stop=(d == NCHUNK - 1 and s == NS - 1))
                    nc.tensor.matmul(out=pool_hi[:], lhsT=oh[:, 128:G], rhs=h2e[:],
                                     start=first, stop=(d == NCHUNK - 1 and s == NS - 1))
                    first = False

            pl = res.tile([128, 33], F32, tag="pl")
            ph = res.tile([128, 33], F32, tag="ph")
            nc.vector.tensor_copy(out=pl[:], in_=pool_lo[:])
            nc.vector.tensor_copy(out=ph[:], in_=pool_hi[:])
            nc.sync.dma_start(out=t_pool[0:128, :], in_=pl[:])
            nc.sync.dma_start(out=t_pool[128:G, :], in_=ph[:])
            nc.gpsimd.collective_compute(
                "AllReduce", mybir.AluOpType.add, replica_groups=groups,
                ins=[t_pool.ap().opt()], outs=[t_poolr.ap().opt()])

            # ---------- mean + MLP head (every core computes the same y) ----------
            for half in range(2):
                pr = res.tile([128, 33], F32, tag=f"pr{half}")
                nc.sync.dma_start(out=pr[:], in_=t_poolr[half * 128:(half + 1) * 128, :])
                cnt = res.tile([128, 1], F32, tag=f"cnt{half}")
                nc.vector.tensor_scalar_max(out=cnt[:], in0=pr[:, 32:33], scalar1=1.0)
                nc.vector.reciprocal(out=cnt[:], in_=cnt[:])
                mean = res.tile([128, HID], F32, tag=f"mean{half}")
                nc.vector.tensor_scalar_mul(out=mean[:], in0=pr[:, 0:HID], scalar1=cnt[:])
                mT_ps = psum.tile([HID, 128], F32, tag="pre")
                nc.tensor.transpose(out=mT_ps[:], in_=mean[:], identity=ident[:])
                mT = res.tile([HID, 128], F32, tag=f"mT{half}")
                nc.vector.tensor_copy(out=mT[:], in_=mT_ps[:])
                hc_ps = psum.tile([128, C1], F32, tag="pre")
                nc.tensor.matmul(out=hc_ps[:], lhsT=mT[:], rhs=Wc1sb[:], start=True, stop=True)
                hc = res.tile([128, C1], F32, tag=f"hc{half}")
                nc.vector.tensor_add(out=hc[:], in0=hc_ps[:], in1=bc1b[:])
                nc.scalar.activation(out=hc[:], in_=hc[:], func=AF.Relu)
                hT_ps = psum.tile([C1, 128], F32, tag="pre")
                nc.tensor.transpose(out=hT_ps[:], in_=hc[:], identity=ident[:])
                hT = res.tile([C1, 128], F32, tag=f"hT{half}")
                nc.vector.tensor_copy(out=hT[:], in_=hT_ps[:])
                o_ps = psum.tile([128, 1], F32, tag="pre")
                nc.tensor.matmul(out=o_ps[:], lhsT=hT[:], rhs=Wc2sb[:], start=True, stop=True)
                ob = res.tile([128, 1], F32, tag=f"ob{half}")
                nc.vector.tensor_add(out=ob[:], in0=o_ps[:], in1=bc2b[:])
                nc.scalar.activation(out=ob[:], in_=ob[:], func=AF.Sigmoid)
                nc.sync.dma_start(out=t_y[half * 128:(half + 1) * 128, :], in_=ob[:])

    nc.compile()
    if split:
        _split_sync_waits(nc)
    return nc


_PROG_CACHE = {}

# test-harness knobs (harness never sets these; defaults keep grading path)
TRACE = False
TRACE_DIR = None
LAST = None


def kernel(**inputs):
    x = np.asarray(inputs["x"]).astype(np.int64).reshape(-1)
    ei = np.asarray(inputs["edge_index"]).astype(np.int64)
    batch = np.asarray(inputs["batch"]).astype(np.int64).reshape(-1)
    emb = np.asarray(inputs["emb"], np.float32)
    W1 = np.asarray(inputs["W1"], np.float32)
    b1 = np.asarray(inputs["b1"], np.float32).reshape(1, -1)
    W2 = np.asarray(inputs["W2"], np.float32)
    b2 = np.asarray(inputs["b2"], np.float32).reshape(1, -1)
    Wc1 = np.asarray(inputs["Wc1"], np.float32)
    bc1 = np.asarray(inputs["bc1"], np.float32).reshape(1, -1)
    Wc2 = np.asarray(inputs["Wc2"], np.float32)
    bc2 = np.asarray(inputs["bc2"], np.float32).reshape(1, -1)

    plan, nslots, per_core = _shard_edges(ei[0], ei[1])

    key = (nslots, tuple((s, d, lo, hi, tuple(sp)) for (s, d, lo, hi, sp) in plan))
    if key not in _PROG_CACHE:
        _PROG_CACHE[key] = _build_program(plan, nslots)
    nc = _PROG_CACHE[key]

    # node-id table in padded row space (row = (n//CH_REAL)*CH_PAD + n%CH_REAL)
    ids_pad = np.zeros(NROW, np.int64)
    rows = (np.arange(N) // CH_REAL) * CH_PAD + (np.arange(N) % CH_REAL)
    ids_pad[rows] = x
    ids16 = _wrap16(ids_pad.astype(np.int16))

    # batch values for each core's final-phase rows: core c, chunk d,
    # subtile s, partition p -> chunk-local row FSLICE*c + s*128 + p
    iota256 = np.tile(np.arange(G, dtype=np.float32), (128, 1))
    ident128 = np.eye(128, dtype=np.float32)

    loc = np.arange(FSLICE)
    in_maps = []
    for c in range(NCORES):
        fl = FSLICE * c + loc          # chunk-local rows this core handles
        fidx = _wrap16(fl.astype(np.int16))
        bvals = np.full((NCHUNK, FSLICE), -1.0, np.float32)
        for d in range(NCHUNK):
            gl = fl.copy()
            real = gl < CH_REAL
            n_global = d * CH_REAL + gl
            ok = real & (n_global < N)
            bvals[d, ok] = batch[n_global[ok]]
        # [128, NCHUNK*FSLICE/128] with col d*NS + s at partition p = row s*128+p
        barr = bvals.reshape(NCHUNK, FSLICE // 128, 128).transpose(2, 0, 1).reshape(
            128, NCHUNK * (FSLICE // 128))
        in_maps.append(dict(
            ids16=ids16,
            batchf=np.ascontiguousarray(barr),
            gidx=_wrap16(per_core[c]["gidx"]),
            sidx=_wrap16(per_core[c]["sidx"]),
            fidx=fidx,
            emb=emb, W1=W1, b1=b1, W2=W2, b2=b2,
            Wc1=Wc1, bc1=bc1, Wc2=Wc2, bc2=bc2,
            iota256=iota256, ident128=ident128,
        ))

    kw = {}
    if TRACE:
        kw = dict(trace=True, tmpdir=TRACE_DIR,
                  trace_cores=list(range(NCORES)))
    res = run_bass_kernel_spmd(nc, in_maps, core_ids=list(range(NCORES)), **kw)
    globals()["LAST"] = res
    return res.results[0]["y"].astype(np.float32)



# revision 19
# speedup vs baseline: 3832.6697x; 3.6493x over previous
"""Trainium2 Bass kernel for a 2-layer GCN + global mean pool + MLP head.

Distribution (8 NeuronCores): dst-sharded edge parallel. Each core owns a
contiguous range of 12544 destination nodes (98 blocks of 128) and receives
every edge (plus self-loops) whose dst falls in its range, sorted by
(dst-block, src-chunk, src). Segment sums are computed with one-hot matmuls
into PSUM per dst block -- no scatter-adds, no races, no accumulator
AllReduce.

Layer 1 exploits the embedding bottleneck: messages are c_src*embW1[ids[src]]
so the gather reads the 200-row emb@W1 table directly with host-computed
ids[src] indices (no N-row t1 table, no deg pass: deg/c come from the host,
which already sorts all edges anyway). Layer 2 gathers from the u2 table
(u2 = c*(h1@W2)), built shard-wise and AllGathered once. The only other
collective is an AllReduce of the [256,32] pooled tensor.

SWDGE gathers are issued as a few thousand-index ops per strip of dst blocks
(994ns fixed overhead per op amortized), unchained: the GPSIMD ucode
backpressures on descriptor-ring space.
"""

import numpy as np

import concourse.bacc as bacc
import concourse.mybir as mybir
import concourse.tile as tile
from concourse.bass_utils import run_bass_kernel_spmd

# ---- problem geometry (hardcoded per task contract) ----
N = 100000
E = 1000000
G = 256
NTYPES = 200
EMB = 64
HID = 32
C1 = 16
NCORES = 8

BLK = 128                       # dst block size
NPAD = 100352                   # 784 blocks * 128
NBLK_CORE = 98                  # blocks per core
NODES_CORE = NBLK_CORE * BLK    # 12544
CHUNK = 25088                   # src chunk (int16-addressable, = NPAD/4)
NCHUNK = 4
UROWS = NCHUNK * CHUNK          # 100352 u2 table rows
STRIP_CAP = 160                 # max staging col-groups per strip

F32 = mybir.dt.float32
I16 = mybir.dt.int16

MAX_WAITS = 1


def _split_sync_waits(nc):
    """walrus TPB codegen encodes at most one sync-wait per instruction;
    split longer wait lists into preceding same-engine nops."""
    n = 0
    for f in nc.m.functions:
        for blk in f.blocks:
            il = blk.instructions
            i = 0
            while i < len(il):
                ins = il[i]
                si = ins.sync_info
                if si is not None and si.on_wait and len(si.on_wait) > MAX_WAITS:
                    w = list(si.on_wait)
                    si.on_wait = w[-MAX_WAITS:]
                    ex = w[:-MAX_WAITS]
                    nops = []
                    for k in range(0, len(ex), MAX_WAITS):
                        p = mybir.InstNoOp(name=f"Wsplit-{n}-{k}")
                        p.engine = ins.engine
                        p.sync_info = mybir.SyncInfo(on_wait=ex[k:k + MAX_WAITS], on_update=[])
                        nops.append(p)
                    for j, p in enumerate(nops):
                        il.insert(i + j, p)
                    i += len(nops)
                    n += 1
                i += 1
    return n


def _wrap16(a):
    """flat int16 index list (len % 16 == 0) -> [128, n/16] wrapped layout,
    replicated across the 8 GPSIMD core groups."""
    w = a.reshape(-1, 16).T.astype(np.int16)
    return np.ascontiguousarray(np.tile(w, (8, 1)))


def _plan_edges(src, dst, ids, cvec):
    """Shard edges by dst across cores; per core sort by (dst block,
    src chunk, src); pad each (block, chunk) run to x128 slots with a
    structure common to all cores (shared SPMD program). Self-loops are NOT
    in the slots -- they are applied at flush time (L1: small embW1[ids[v]]
    gather; L2: direct read of the core's own u2 shard).

    Returns (plan, ncols, per_core):
      plan: list of strips; strip = dict(blocks=[b..], ops=[(ch, col0, col1)],
            groups=[(b, col)], colbase, ncols)
      ncols: total staging col-groups per layer
      per_core: dicts with gidx1/gidx2/gidxS (int16 wrapped), dstval/cval
            ([128, ncols] f32)
    """
    s = np.asarray(src)
    d = np.asarray(dst)

    owner = d // NODES_CORE
    blk_l = (d % NODES_CORE) // BLK          # local block 0..97
    ch = s // CHUNK
    # per (core, block, chunk) run lengths
    key = (owner * NBLK_CORE + blk_l) * NCHUNK + ch
    cnt = np.bincount(key, minlength=NCORES * NBLK_CORE * NCHUNK)
    cnt = cnt.reshape(NCORES, NBLK_CORE, NCHUNK)
    mx = cnt.max(axis=0)                     # [98, 4]
    L = np.where(mx > 0, -(-mx // BLK) * BLK, 0)   # padded run len, x128

    # strips: consecutive blocks, capped col-groups
    blk_cols = L.sum(axis=1) // BLK          # cols per block
    strips = []
    cur, cur_cols = [], 0
    for b in range(NBLK_CORE):
        if cur and cur_cols + blk_cols[b] > STRIP_CAP:
            strips.append(cur)
            cur, cur_cols = [], 0
        cur.append(b)
        cur_cols += blk_cols[b]
    if cur:
        strips.append(cur)

    # slot layout: strip-major; within strip chunk-major, then block
    plan = []
    col = 0
    slot_of = np.zeros((NBLK_CORE, NCHUNK), np.int64)   # slot base per run
    for blocks in strips:
        colbase = col
        ops = []
        for c in range(NCHUNK):
            c0 = col
            for b in blocks:
                slot_of[b, c] = col * BLK
                col += L[b, c] // BLK
            if col > c0:
                ops.append((c, c0, col))
        groups = []
        for c in range(NCHUNK):
            for b in blocks:
                for g in range(L[b, c] // BLK):
                    groups.append((b, slot_of[b, c] // BLK + g))
        plan.append(dict(blocks=blocks, ops=ops, groups=groups,
                         colbase=colbase, ncols=col - colbase))
    ncols = col
    nslots = ncols * BLK

    # per-core slot arrays
    order = np.lexsort((s, ch, blk_l, owner))
    so, do_, blo, cho, own_o = s[order], d[order], blk_l[order], ch[order], owner[order]
    per_core = []
    for core in range(NCORES):
        m = own_o == core
        es, ed, eb, ec = so[m], do_[m], blo[m], cho[m]
        gidx1 = np.zeros(nslots, np.int16)           # ids[src] into embW1
        gidx2 = np.zeros(nslots, np.int16)           # src chunk-local into u2
        dstval = np.full(nslots, -1.0, np.float32)   # dst offset in block
        cval = np.zeros(nslots, np.float32)          # c[src]
        # place runs: edges already sorted by (block, chunk, src)
        rk = eb * NCHUNK + ec
        bounds = np.searchsorted(rk, np.arange(NBLK_CORE * NCHUNK + 1))
        for b in range(NBLK_CORE):
            for c in range(NCHUNK):
                lo, hi = bounds[b * NCHUNK + c], bounds[b * NCHUNK + c + 1]
                if hi == lo:
                    continue
                base = slot_of[b, c]
                run_s = es[lo:hi]
                gidx1[base:base + hi - lo] = ids[run_s].astype(np.int16)
                gidx2[base:base + hi - lo] = (run_s - c * CHUNK).astype(np.int16)
                dstval[base:base + hi - lo] = (ed[lo:hi] % BLK).astype(np.float32)
                cval[base:base + hi - lo] = cvec[run_s]
        # self gather: ids of the core's own nodes, node order
        nodes = core * NODES_CORE + np.arange(NODES_CORE)
        gS = np.zeros(NODES_CORE, np.int16)
        rm = nodes < N
        gS[rm] = ids[nodes[rm]].astype(np.int16)
        per_core.append(dict(
            gidx1=_wrap16(gidx1), gidx2=_wrap16(gidx2), gidxS=_wrap16(gS),
            dstval=np.ascontiguousarray(dstval.reshape(ncols, BLK).T),
            cval=np.ascontiguousarray(cval.reshape(ncols, BLK).T),
        ))
    return plan, ncols, per_core


def _build_program(plan, ncols):
    nc = bacc.Bacc("TRN2", target_bir_lowering=False, debug=False, num_devices=NCORES)
    AF = mybir.ActivationFunctionType
    groups8 = [list(range(NCORES))]
    maxcols = max(st["ncols"] for st in plan)
    maxblk = max(len(st["blocks"]) for st in plan)

    t_gidx1 = nc.dram_tensor("gidx1", [128, ncols * BLK // 16], I16, kind="ExternalInput")
    t_gidx2 = nc.dram_tensor("gidx2", [128, ncols * BLK // 16], I16, kind="ExternalInput")
    t_gidxS = nc.dram_tensor("gidxS", [128, NODES_CORE // 16], I16, kind="ExternalInput")
    t_dstval = nc.dram_tensor("dstval", [128, ncols], F32, kind="ExternalInput")
    t_cval = nc.dram_tensor("cval", [128, ncols], F32, kind="ExternalInput")
    t_cblk = nc.dram_tensor("cblk", [128, NBLK_CORE], F32, kind="ExternalInput")
    t_cblk2 = nc.dram_tensor("cblk2", [128, NBLK_CORE], F32, kind="ExternalInput")
    t_bval = nc.dram_tensor("bval", [128, NBLK_CORE], F32, kind="ExternalInput")
    t_icnt = nc.dram_tensor("icnt", [128, 2], F32, kind="ExternalInput")
    t_emb = nc.dram_tensor("emb", [NTYPES, EMB], F32, kind="ExternalInput")
    t_W1 = nc.dram_tensor("W1", [EMB, HID], F32, kind="ExternalInput")
    t_b1 = nc.dram_tensor("b1", [1, HID], F32, kind="ExternalInput")
    t_W2 = nc.dram_tensor("W2", [HID, HID], F32, kind="ExternalInput")
    t_b2 = nc.dram_tensor("b2", [1, HID], F32, kind="ExternalInput")
    t_Wc1 = nc.dram_tensor("Wc1", [HID, C1], F32, kind="ExternalInput")
    t_bc1 = nc.dram_tensor("bc1", [1, C1], F32, kind="ExternalInput")
    t_Wc2 = nc.dram_tensor("Wc2", [C1, 1], F32, kind="ExternalInput")
    t_bc2 = nc.dram_tensor("bc2", [1, 1], F32, kind="ExternalInput")
    t_iota128 = nc.dram_tensor("iota128", [128, 128], F32, kind="ExternalInput")
    t_iota256 = nc.dram_tensor("iota256", [128, G], F32, kind="ExternalInput")
    t_ident = nc.dram_tensor("ident128", [128, 128], F32, kind="ExternalInput")
    t_y = nc.dram_tensor("y", [G, 1], F32, kind="ExternalOutput")

    t_embw = nc.dram_tensor("embw", [256, EMB], F32)
    t_u2own = nc.dram_tensor("u2own", [NODES_CORE, EMB], F32)
    t_u2full = nc.dram_tensor("u2full", [UROWS, EMB], F32, addr_space="Shared")
    t_pool = nc.dram_tensor("pooled", [G, HID], F32)
    t_poolr = nc.dram_tensor("pooledr", [G, HID], F32, addr_space="Shared")

    with tile.TileContext(nc) as tc:
        with (
            tc.tile_pool(name="res", bufs=1) as res,
            tc.tile_pool(name="stage", bufs=2) as stage,
            tc.tile_pool(name="ptile", bufs=3) as ptile,
            tc.tile_pool(name="fine", bufs=3) as fine,
            tc.tile_pool(name="ps", bufs=2, space="PSUM") as psum,
            tc.tile_pool(name="zacc", bufs=2, space="PSUM") as zpool,
            tc.tile_pool(name="poolacc", bufs=1, space="PSUM") as poolacc,
        ):
            # ---------- residents ----------
            gidx1 = res.tile([128, ncols * BLK // 16], I16)
            nc.sync.dma_start(out=gidx1[:], in_=t_gidx1[:, :])
            gidx2 = res.tile([128, ncols * BLK // 16], I16)
            nc.sync.dma_start(out=gidx2[:], in_=t_gidx2[:, :])
            gidxS = res.tile([128, NODES_CORE // 16], I16)
            nc.sync.dma_start(out=gidxS[:], in_=t_gidxS[:, :])
            dstval = res.tile([128, ncols], F32)
            nc.sync.dma_start(out=dstval[:], in_=t_dstval[:, :])
            cval = res.tile([128, ncols], F32)
            nc.sync.dma_start(out=cval[:], in_=t_cval[:, :])
            cblk = res.tile([128, NBLK_CORE], F32)
            nc.sync.dma_start(out=cblk[:], in_=t_cblk[:, :])
            cblk2 = res.tile([128, NBLK_CORE], F32)
            nc.sync.dma_start(out=cblk2[:], in_=t_cblk2[:, :])
            bval = res.tile([128, NBLK_CORE], F32)
            nc.sync.dma_start(out=bval[:], in_=t_bval[:, :])
            icnt = res.tile([128, 2], F32)
            nc.sync.dma_start(out=icnt[:], in_=t_icnt[:, :])
            iota128 = res.tile([128, 128], F32)
            nc.sync.dma_start(out=iota128[:], in_=t_iota128[:, :])
            iota256 = res.tile([128, G], F32)
            nc.sync.dma_start(out=iota256[:], in_=t_iota256[:, :])
            ident = res.tile([128, 128], F32)
            nc.sync.dma_start(out=ident[:], in_=t_ident[:, :])
            onesP = res.tile([1, 128], F32)
            nc.vector.memset(onesP[:], 1.0)
            W1sb = res.tile([EMB, HID], F32)
            nc.sync.dma_start(out=W1sb[:], in_=t_W1[:, :])
            W2sb = res.tile([HID, HID], F32)
            nc.sync.dma_start(out=W2sb[:], in_=t_W2[:, :])
            Wc1sb = res.tile([HID, C1], F32)
            nc.sync.dma_start(out=Wc1sb[:], in_=t_Wc1[:, :])
            Wc2sb = res.tile([C1, 1], F32)
            nc.sync.dma_start(out=Wc2sb[:], in_=t_Wc2[:, :])

            def bcast_row(t_dram, w, nm):
                row = res.tile([1, w], F32, tag=f"row_{nm}")
                nc.sync.dma_start(out=row[:], in_=t_dram[:, :])
                p = psum.tile([128, w], F32, tag="pre")
                nc.tensor.matmul(out=p[:], lhsT=onesP[:], rhs=row[:], start=True, stop=True)
                out = res.tile([128, w], F32, tag=f"bc_{nm}")
                nc.vector.tensor_copy(out=out[:], in_=p[:])
                return out

            b1b = bcast_row(t_b1, HID, "b1")
            b2b = bcast_row(t_b2, HID, "b2")
            bc1b = bcast_row(t_bc1, C1, "bc1")
            bc2b = bcast_row(t_bc2, 1, "bc2")

            # ---------- embW1 = emb @ W1 ----------
            emb_lo = res.tile([128, EMB], F32)
            nc.sync.dma_start(out=emb_lo[:], in_=t_emb[0:128, :])
            ps1 = psum.tile([EMB, 128], F32, tag="pre")
            nc.tensor.transpose(out=ps1[:], in_=emb_lo[:], identity=ident[:])
            embT_lo = res.tile([EMB, 128], F32)
            nc.vector.tensor_copy(out=embT_lo[:], in_=ps1[:])
            emb_hi = res.tile([72, EMB], F32)
            nc.sync.dma_start(out=emb_hi[:], in_=t_emb[128:200, :])
            ps2 = psum.tile([EMB, 72], F32, tag="pre")
            nc.tensor.transpose(out=ps2[:], in_=emb_hi[:], identity=ident[0:72, 0:72])
            embT_hi = res.tile([EMB, 72], F32)
            nc.vector.tensor_copy(out=embT_hi[:], in_=ps2[:])
            ew_ps = psum.tile([128, HID], F32, tag="pre")
            nc.tensor.matmul(out=ew_ps[:], lhsT=embT_lo[:], rhs=W1sb[:], start=True, stop=True)
            ew_lo = res.tile([128, HID], F32)
            nc.vector.tensor_copy(out=ew_lo[:], in_=ew_ps[:])
            nc.sync.dma_start(out=t_embw[0:128, 0:HID], in_=ew_lo[:])
            ew_ps2 = psum.tile([72, HID], F32, tag="pre")
            nc.tensor.matmul(out=ew_ps2[:], lhsT=embT_hi[:], rhs=W1sb[:], start=True, stop=True)
            ew_hi = res.tile([72, HID], F32)
            nc.vector.tensor_copy(out=ew_hi[:], in_=ew_ps2[:])
            nc.sync.dma_start(out=t_embw[128:200, 0:HID], in_=ew_hi[:])

            poolboth = poolacc.tile([128, 2 * HID], F32)
            pool_lo = poolboth[:, 0:HID]
            pool_hi = poolboth[:, HID:2 * HID]
            first_pool = [True]

            def edge_layer(layer):
                for st in plan:
                    cb = st["colbase"]
                    sn = st["ncols"]
                    b0, nblk = st["blocks"][0], len(st["blocks"])
                    stg = stage.tile([128, maxcols, EMB], F32, tag="stg")
                    gidx = gidx1 if layer == 1 else gidx2
                    for (c, c0, c1) in st["ops"]:
                        n = (c1 - c0) * BLK
                        if layer == 1:
                            tbl = t_embw[0:256, :]
                        else:
                            tbl = t_u2full[c * CHUNK:(c + 1) * CHUNK, :]
                        nc.gpsimd.dma_gather(
                            stg[:, c0 - cb:c1 - cb, :], tbl,
                            gidx[:, c0 * BLK // 16:c1 * BLK // 16],
                            n, n, EMB, single_packet=False)
                    # self term: L1 gathers embW1[ids[v]]; L2 loads own u2 rows
                    selfstg = stage.tile([128, maxblk, EMB], F32, tag="selfstg")
                    if layer == 1:
                        nc.gpsimd.dma_gather(
                            selfstg[:, 0:nblk, :], t_embw[0:256, :],
                            gidxS[:, b0 * BLK // 16:(b0 + nblk) * BLK // 16],
                            nblk * BLK, nblk * BLK, EMB, single_packet=False)
                        cvb = cval[:, cb:cb + sn, None].to_broadcast([128, sn, HID])
                        nc.vector.tensor_mul(
                            out=stg[:, 0:sn, 0:HID], in0=stg[:, 0:sn, 0:HID], in1=cvb)
                    else:
                        nc.sync.dma_start(
                            out=selfstg[:, 0:nblk, 0:HID],
                            in_=t_u2own[b0 * BLK:(b0 + nblk) * BLK, :]
                                .rearrange("(j p) e -> p j e", p=128)[:, :, 0:HID])
                    # per-block accumulation: all blocks of the strip share
                    # one PSUM bank, 32-col slices each
                    zbig = zpool.tile([128, 512], F32, tag="zbig")
                    jb = {b: j for j, b in enumerate(st["blocks"])}
                    ng = {}
                    for (b, g) in st["groups"]:
                        ng[b] = ng.get(b, 0) + 1
                    seen = {}
                    for (b, g) in st["groups"]:
                        P = ptile.tile([128, 128], F32, tag="P")
                        nc.vector.tensor_tensor(
                            out=P[:], in0=dstval[:, g, None].to_broadcast([128, 128]),
                            in1=iota128[:], op=mybir.AluOpType.is_equal)
                        seen[b] = seen.get(b, 0) + 1
                        nc.tensor.matmul(
                            out=zbig[:, jb[b] * HID:(jb[b] + 1) * HID],
                            lhsT=P[:], rhs=stg[:, g - cb, 0:HID],
                            start=(seen[b] == 1), stop=(seen[b] == ng[b]))
                    # flush blocks: y = relu(c*z + cself*selfrow + b)
                    for j, b in enumerate(st["blocks"]):
                        y = fine.tile([128, HID], F32, tag="fl_y")
                        nc.vector.tensor_scalar_mul(
                            out=y[:], in0=zbig[:, j * HID:(j + 1) * HID],
                            scalar1=cblk[:, b:b + 1])
                        sr = fine.tile([128, HID], F32, tag="fl_sr")
                        nc.vector.tensor_scalar_mul(
                            out=sr[:], in0=selfstg[:, j, 0:HID],
                            scalar1=(cblk2 if layer == 1 else cblk)[:, b:b + 1])
                        nc.vector.tensor_add(out=y[:], in0=y[:], in1=sr[:])
                        nc.vector.tensor_add(out=y[:], in0=y[:], in1=(b1b if layer == 1 else b2b)[:])
                        nc.scalar.activation(out=y[:], in_=y[:], func=AF.Relu)
                        if layer == 1:
                            tp = psum.tile([HID, 128], F32, tag="fl")
                            nc.tensor.transpose(out=tp[:], in_=y[:], identity=ident[:])
                            yT = fine.tile([HID, 128], F32, tag="fl_yT")
                            nc.vector.tensor_copy(out=yT[:], in_=tp[:])
                            mm = psum.tile([128, HID], F32, tag="fl")
                            nc.tensor.matmul(out=mm[:], lhsT=yT[:], rhs=W2sb[:], start=True, stop=True)
                            u2 = fine.tile([128, HID], F32, tag="fl_u2")
                            nc.vector.tensor_scalar_mul(out=u2[:], in0=mm[:], scalar1=cblk[:, b:b + 1])
                            nc.sync.dma_start(
                                out=t_u2own[b * BLK:(b + 1) * BLK, 0:HID], in_=u2[:])
                        else:
                            oh = fine.tile([128, G], F32, tag="fl_oh")
                            nc.vector.tensor_tensor(
                                out=oh[:], in0=bval[:, b, None].to_broadcast([128, G]),
                                in1=iota256[:], op=mybir.AluOpType.is_equal)
                            fp = first_pool[0]
                            lp = (st is plan[-1]) and (b == st["blocks"][-1])
                            nc.tensor.matmul(out=pool_lo[:], lhsT=oh[:, 0:128], rhs=y[:],
                                             start=fp, stop=lp)
                            nc.tensor.matmul(out=pool_hi[:], lhsT=oh[:, 128:G], rhs=y[:],
                                             start=fp, stop=lp)
                            first_pool[0] = False

            # ---------- L1 ----------
            edge_layer(1)
            nc.gpsimd.collective_compute(
                "AllGather", mybir.AluOpType.bypass, replica_groups=groups8,
                ins=[t_u2own.ap().opt()], outs=[t_u2full[0:NCORES * NODES_CORE, :].opt()])

            # ---------- L2 + pool ----------
            edge_layer(2)

            pl = res.tile([128, HID], F32, tag="pl")
            ph = res.tile([128, HID], F32, tag="ph")
            nc.vector.tensor_copy(out=pl[:], in_=pool_lo[:])
            nc.vector.tensor_copy(out=ph[:], in_=pool_hi[:])
            nc.sync.dma_start(out=t_pool[0:128, :], in_=pl[:])
            nc.sync.dma_start(out=t_pool[128:G, :], in_=ph[:])
            nc.gpsimd.collective_compute(
                "AllReduce", mybir.AluOpType.add, replica_groups=groups8,
                ins=[t_pool.ap().opt()], outs=[t_poolr.ap().opt()])

            # ---------- mean + MLP head (every core computes the same y) ----------
            for half in range(2):
                pr = res.tile([128, HID], F32, tag=f"pr{half}")
                nc.sync.dma_start(out=pr[:], in_=t_poolr[half * 128:(half + 1) * 128, :])
                mean = res.tile([128, HID], F32, tag=f"mean{half}")
                nc.vector.tensor_scalar_mul(out=mean[:], in0=pr[:], scalar1=icnt[:, half:half + 1])
                mT_ps = psum.tile([HID, 128], F32, tag="pre")
                nc.tensor.transpose(out=mT_ps[:], in_=mean[:], identity=ident[:])
                mT = res.tile([HID, 128], F32, tag=f"mT{half}")
                nc.vector.tensor_copy(out=mT[:], in_=mT_ps[:])
                hc_ps = psum.tile([128, C1], F32, tag="pre")
                nc.tensor.matmul(out=hc_ps[:], lhsT=mT[:], rhs=Wc1sb[:], start=True, stop=True)
                hc = res.tile([128, C1], F32, tag=f"hc{half}")
                nc.vector.tensor_add(out=hc[:], in0=hc_ps[:], in1=bc1b[:])
                nc.scalar.activation(out=hc[:], in_=hc[:], func=AF.Relu)
                hT_ps = psum.tile([C1, 128], F32, tag="pre")
                nc.tensor.transpose(out=hT_ps[:], in_=hc[:], identity=ident[:])
                hT = res.tile([C1, 128], F32, tag=f"hT{half}")
                nc.vector.tensor_copy(out=hT[:], in_=hT_ps[:])
                o_ps = psum.tile([128, 1], F32, tag="pre")
                nc.tensor.matmul(out=o_ps[:], lhsT=hT[:], rhs=Wc2sb[:], start=True, stop=True)
                ob = res.tile([128, 1], F32, tag=f"ob{half}")
                nc.vector.tensor_add(out=ob[:], in0=o_ps[:], in1=bc2b[:])
                nc.scalar.activation(out=ob[:], in_=ob[:], func=AF.Sigmoid)
                nc.sync.dma_start(out=t_y[half * 128:(half + 1) * 128, :], in_=ob[:])

    nc.compile()
    _split_sync_waits(nc)
    return nc


_PROG_CACHE = {}

# test-harness knobs (harness never sets these; defaults keep grading path)
TRACE = False
TRACE_DIR = None
TRACE_CORES = None
LAST = None


def kernel(**inputs):
    x = np.asarray(inputs["x"]).astype(np.int64).reshape(-1)
    ei = np.asarray(inputs["edge_index"]).astype(np.int64)
    batch = np.asarray(inputs["batch"]).astype(np.int64).reshape(-1)
    emb = np.asarray(inputs["emb"], np.float32)
    W1 = np.asarray(inputs["W1"], np.float32)
    b1 = np.asarray(inputs["b1"], np.float32).reshape(1, -1)
    W2 = np.asarray(inputs["W2"], np.float32)
    b2 = np.asarray(inputs["b2"], np.float32).reshape(1, -1)
    Wc1 = np.asarray(inputs["Wc1"], np.float32)
    bc1 = np.asarray(inputs["bc1"], np.float32).reshape(1, -1)
    Wc2 = np.asarray(inputs["Wc2"], np.float32)
    bc2 = np.asarray(inputs["bc2"], np.float32).reshape(1, -1)

    # degree (incl self-loop) and norm from host: the planner sorts every
    # edge anyway, so deg is a byproduct of input distribution
    deg = np.bincount(ei[1], minlength=N).astype(np.float64) + 1.0
    cvec = (1.0 / np.sqrt(deg)).astype(np.float32)

    plan, ncols, per_core = _plan_edges(ei[0], ei[1], x, cvec)

    key = (ncols, tuple((tuple(st["blocks"]), tuple(st["ops"])) for st in plan))
    if key not in _PROG_CACHE:
        _PROG_CACHE[key] = _build_program(plan, ncols)
    nc = _PROG_CACHE[key]

    iota128 = np.tile(np.arange(128, dtype=np.float32), (128, 1))
    iota256 = np.tile(np.arange(G, dtype=np.float32), (128, 1))
    ident128 = np.eye(128, dtype=np.float32)
    counts = np.bincount(batch, minlength=G).astype(np.float32)
    icnt = (1.0 / np.maximum(counts, 1.0)).reshape(2, 128).T.copy()  # [128, 2]

    in_maps = []
    for c in range(NCORES):
        nodes = c * NODES_CORE + np.arange(NODES_CORE)
        real = nodes < N
        cb = np.zeros(NODES_CORE, np.float32)
        cb[real] = cvec[nodes[real]]
        bv = np.full(NODES_CORE, -1.0, np.float32)
        bv[real] = batch[nodes[real]]
        in_maps.append(dict(
            gidx1=per_core[c]["gidx1"], gidx2=per_core[c]["gidx2"],
            gidxS=per_core[c]["gidxS"],
            dstval=per_core[c]["dstval"], cval=per_core[c]["cval"],
            cblk=np.ascontiguousarray(cb.reshape(NBLK_CORE, BLK).T),
            cblk2=np.ascontiguousarray((cb * cb).reshape(NBLK_CORE, BLK).T),
            bval=np.ascontiguousarray(bv.reshape(NBLK_CORE, BLK).T),
            icnt=icnt,
            emb=emb, W1=W1, b1=b1, W2=W2, b2=b2,
            Wc1=Wc1, bc1=bc1, Wc2=Wc2, bc2=bc2,
            iota128=iota128, iota256=iota256, ident128=ident128,
        ))

    kw = {}
    if TRACE:
        kw = dict(trace=True, tmpdir=TRACE_DIR,
                  trace_cores=TRACE_CORES or [0])
    res = run_bass_kernel_spmd(nc, in_maps, core_ids=list(range(NCORES)), **kw)
    globals()["LAST"] = res
    return res.results[0]["y"].astype(np.float32)
